# revision 52
# baseline (speedup 1.0000x reference)
"""Causal self-attention Trainium2 kernel (8 NeuronCores, SPMD) — v3.

Sharding: 8 cores = 4 batches x 2 head-groups. Each core computes, for its
(batch b, head-group g): Q/K/V projections restricted to g's 8 heads
(column-parallel), causal attention for those heads, and the partial output
projection ctx_g @ Wo[g rows] (row-parallel). Host sums the two partials per
batch and adds the bias terms (bv @ Wo + bo).

v3 changes vs v2 (v2 measured 634 us, PE busy 91%, ~58 us PE idle; v3
measures ~627.5 us, P1/V phase fully PE-dense at 99-100%):
- P1/P1b fusion: Q/K rounds and V seq-tile rounds share one 6-bank PSUM pool
  with V rounds interleaved between Q/K rounds, so there is no P1->P1b
  transition stall. Head 0 runs a special 8-bank q+k-interleaved round
  (borrowing the p4s banks, idle during P1) so its chunk consumption
  (1.7us/chunk) stays above the x DMA delivery rate (~1.4us/chunk) — the PE
  is work-limited from the first matmul on.
- DMA head: x moves as 16 full chunks on SWDGE (issue-rate bound), first
  strips by need-order; deadlock-aware strip ordering (h1 strips after all x
  issues: they reuse h0's slots and would block the gpsimd queue).
- Head-tail software pipelining: each head's closing ops are deferred into
  the NEXT head's kt loop, split in two (final PV + pv copy at kt==3;
  normalizer matmul + reciprocal + ctx mul at kt==5), so the PE never sits
  on the score->exp->acc dependency chains at head boundaries.
- Deficit-paced out-projection interleave: scalar exp is the local pacer in
  later q-blocks; p4 pops are paced by the per-q-block scalar-PE deficit.
  The qb3 Q projection of heads 5-7 is deferred out of P1 (re-reading x cols
  and wq strips from DRAM) and injected as filler for qb0, which otherwise
  has no backlog.
- Engine placement tuned by trace: exp + Q/K bias on scalar only, PSUM
  evictions and all normalization on DVE (cross-engine offloads of
  chain-adjacent ops to gpsimd measurably backfire on semaphore latency),
  out stores on the sync HWDGE ring (scalar helps only in the drain), SWDGE
  queues empty at kernel end (no long GpSimd drain).
- Output is written in bf16 (host upcasts and sums the two group partials).
"""

import sys

sys.path.insert(0, "/opt/trn_rl_repo")

from collections import deque
from contextlib import ExitStack

import numpy as np

import concourse.bass as bass
import concourse.tile as tile
from concourse import mybir
from concourse.bass_utils import run_bass_kernel_spmd

BF16 = mybir.dt.bfloat16
F32 = mybir.dt.float32
NP_BF16 = mybir.dt.np(BF16)

# Problem constants (hardcoded per contract).
B = 4          # batch
S = 2048       # sequence length
DM = 2048      # d_model
H = 16         # total heads
HD = 128       # head dim
G = 2          # head groups (tensor parallel degree)
NHL = H // G   # local heads per core
DHL = NHL * HD # local head dims
NCORES = 8
P = 128        # partitions
FD = 512       # matmul moving free dim (one PSUM bank of f32)
NKC = DM // P  # contraction chunks for projections
NST = S // P   # seq tiles (k tiles)
NQB = S // FD  # 512-wide q blocks
SCALE = 1.0 / float(np.sqrt(HD))
MASK_VAL = -1e30

# Per-head scalar-vs-PE deficit (ns) per q-block: how much PE filler each
# head's attention needs so the PE does not outrun the exp stream and stall
# on the score->exp->PV chain. Statically derived from measured ACTIVATE
# cost (259 ns + 0.836 ns/col) vs matmul cost (213 ns / 512 cols).
DEFICIT_NS = {0: 2100.0, 1: 3000.0, 2: 3400.0, 3: 5000.0}
MM_COST = 426.0  # one p4 micro-op = 2 matmuls of 512 rows
# Heads whose Q projection for the last q-block is deferred out of P1 and
# injected as PE filler during qb0 (which otherwise has no pop backlog).
DEFQ_HEADS = (5, 6, 7)

_WAIT_EXEMPT = {
    "NoOp",
    "EventSemaphore",
    "UnconditionalBranch",
    "RegisterMove",
    "TileRelease",
}


def _fix_sync_waits(nc, max_waits=1):
    """Hoist extra sync-waits onto single-wait NoOps on the issuing engine.

    Several walrus instruction encodings (PSEUDO_DMA_DIRECT2D, S3_LW, CTRL_NO,
    ...) have a single sync-wait slot and fail codegen with "Too many sync
    wait commands" when Tile attaches more. A NoOp on the same engine
    immediately before the instruction performs the extra wait at the
    sequencer, which is semantically identical.
    """
    f = nc.m.functions[0]
    fixed = 0

    def walk(blocks):
        nonlocal fixed
        for b in blocks:
            il = b.instructions
            i = 0
            while i < len(il):
                inst = il[i]
                si = getattr(inst, "sync_info", None)
                ow = list(si.on_wait) if si is not None and si.on_wait else []
                if inst.opcode not in _WAIT_EXEMPT and len(ow) > max_waits:
                    keep = ow[len(ow) - max_waits :]
                    extra = ow[: len(ow) - max_waits]
                    for j, w in enumerate(extra):
                        nop = mybir.InstNoOp(
                            name=f"{inst.name}_waitfix{j}",
                            engine=inst.engine,
                            ins=[],
                            outs=[],
                            bass_nofuse=True,
                            sync_info=mybir.SyncInfo(on_wait=[w], on_update=[]),
                        )
                        il.insert(i, nop)
                        i += 1
                    inst.sync_info = mybir.SyncInfo(
                        on_wait=keep,
                        on_update=list(si.on_update) if si.on_update else [],
                    )
                    fixed += 1
                i += 1
            walk(getattr(b, "blocks", []) or [])

    walk(f.blocks)
    return fixed


def build_nc(fix_waits=True):
    """Build the single-core Bass program (same program for all 8 cores)."""
    nc = bass.Bass()
    # Inputs are pre-arranged on the host so every DMA line is contiguous.
    # wq/wk are half-strip-major so one [P, NKC//2, P] half is a contiguous
    # 2 KB line per partition (256 B lines are below SDMA line rate).
    xT_d = nc.dram_tensor("xT", [P, NKC, S], BF16, kind="ExternalInput")
    wq_d = nc.dram_tensor("wq", [NHL, 2, P, NKC // 2, P], BF16, kind="ExternalInput")
    wk_d = nc.dram_tensor("wk", [NHL, 2, P, NKC // 2, P], BF16, kind="ExternalInput")
    wv_d = nc.dram_tensor("wv", [P, NKC, DHL], BF16, kind="ExternalInput")
    wo_d = nc.dram_tensor("wo", [P, DHL // P, DM], BF16, kind="ExternalInput")
    bqk_d = nc.dram_tensor("bqk", [P, 2, NHL], F32, kind="ExternalInput")
    out_d = nc.dram_tensor("out", [S, DM], BF16, kind="ExternalOutput")

    with tile.TileContext(nc) as tc:
        # ------------------------- pools (left stack) ---------------------
        es_main = ExitStack()
        consts = es_main.enter_context(tc.tile_pool(name="consts", bufs=1))
        bqk_sb = consts.tile([P, 2, NHL], F32)
        ones_sb = consts.tile([P, P], BF16)
        umask = consts.tile([P, P], F32)

        qkv = es_main.enter_context(tc.tile_pool(name="qkv", bufs=1))
        QT = qkv.tile([P, NHL, S], BF16)
        KT = qkv.tile([P, NHL, S], BF16)

        es_x = ExitStack()
        xpool = es_x.enter_context(tc.tile_pool(name="xpool", bufs=1))
        xT = xpool.tile([P, NKC, S], BF16)

        # ------------------------- pools (right stack) --------------------
        # LIFO close order: strips (end P1) -> wv (end P1b) -> V (end).
        es_v = ExitStack()
        vpool = es_v.enter_context(tc.tile_pool(name="vpool", bufs=1, side="right"))
        V = vpool.tile([P, NST, DHL], BF16)

        es_wv = ExitStack()
        wvpool = es_wv.enter_context(
            tc.tile_pool(name="wvpool", bufs=1, side="right")
        )
        wv_sb = wvpool.tile([P, NKC, DHL], BF16)

        es_strip = ExitStack()
        spool = es_strip.enter_context(
            tc.tile_pool(name="spool", bufs=6, side="right")
        )

        # ------------------------- DMA issue (order = priority) -----------
        # The SWDGE (gpsimd) ring spreads consecutive dma_starts across ~16
        # parallel queue rows. SWDGE issue costs ~0.6us per dma_start; x goes
        # first, split per (chunk, seq-half): 32 issues. The first q/k strips
        # ride the two HWDGE rings (issued by the otherwise idle sync and
        # scalar engines) so the first P1 matmul starts at ~2us.
        strips = {}  # (h, 'q'|'k', half) -> tile

        def load_strip(h, eng):
            for kind, src in (("q", wq_d), ("k", wk_d)):
                for half in range(2):
                    t = spool.tile(
                        [P, NKC // 2, P], BF16, tag="strip", name=f"w{kind}{h}_{half}"
                    )
                    eng.dma_start(out=t[:, :, :], in_=src[h, half, :, :, :])
                    strips[(h, kind, half)] = t

        nc.sync.dma_start(out=bqk_sb[:, :, :], in_=bqk_d[:, :, :])
        # Everything on SWDGE (per-transfer packets spread over all 16 DMA
        # engines; the aggregate ~360 GB/s HBM rate is the wall, so order
        # strictly by need). x moves as FULL chunks (4 KB lines, one issue
        # each). P1's h0 interleaves q AND k per chunk (1.7 us/chunk work vs
        # ~1.4 us/chunk delivery), so the PE is work-limited from the first
        # matmul on.
        def strip_piece(h, kind, half, eng):
            src = wq_d if kind == "q" else wk_d
            t = spool.tile(
                [P, NKC // 2, P], BF16, tag="strip", name=f"w{kind}{h}_{half}"
            )
            eng.dma_start(out=t[:, :, :], in_=src[h, half, :, :, :])
            strips[(h, kind, half)] = t

        strip_piece(0, "q", 0, nc.gpsimd)
        nc.gpsimd.dma_start(out=xT[:, 0, 0 : S // 2], in_=xT_d[:, 0, 0 : S // 2])
        strip_piece(0, "k", 0, nc.gpsimd)
        nc.gpsimd.dma_start(out=xT[:, 0, S // 2 : S], in_=xT_d[:, 0, S // 2 : S])
        for i in range(1, NKC):
            nc.gpsimd.dma_start(out=xT[:, i, :], in_=xT_d[:, i, :])
            if i == 4:
                strip_piece(0, "q", 1, nc.gpsimd)
                strip_piece(0, "k", 1, nc.gpsimd)
        # h1 strips AFTER all x issues: their DMAs reuse h0's strip slots and
        # wait on h0 consumption — anything queued behind them on the gpsimd
        # engine would deadlock against P1's x needs.
        load_strip(1, nc.gpsimd)
        for j in range(0, NKC, 4):
            nc.gpsimd.dma_start(
                out=wv_sb[:, j : j + 4, :], in_=wv_d[:, j : j + 4, :]
            )

        # ------------------------- constants setup ------------------------
        nc.vector.memset(ones_sb[:, :], 1.0)
        # umask[k, q] = 0 if q >= k else MASK_VAL (transposed diagonal block).
        nc.gpsimd.memset(umask[:, :], 0.0)
        nc.gpsimd.affine_select(
            out=umask[:, :],
            in_=umask[:, :],
            compare_op=mybir.AluOpType.is_ge,
            fill=MASK_VAL,
            base=0,
            pattern=[[1, P]],
            channel_multiplier=-1,
        )

        # ------------------------- P1 + P1b: projections -------------------
        # One shared 6-bank PSUM pool for Q/K rounds (4 tiles) and V rounds
        # (2 tiles): V seq-tile rounds are interleaved between Q/K rounds
        # (one per round once x/wv are resident), so there is no P1->P1b
        # transition stall. p4s (banks 6-7, program-lifetime, right stack) is
        # idle during P1 and lends its 2 banks to h0's 8-bank q+k round.
        es_pp = ExitStack()
        ppsum = es_pp.enter_context(tc.tile_pool(name="ppsum", bufs=6, space="PSUM"))
        p4s = es_main.enter_context(
            tc.tile_pool(name="p4s", bufs=2, space="PSUM", side="right")
        )

        # h0: q and k interleaved per chunk, paced to x chunk arrival (the
        # x stream is still in flight; a q-only round would starve the PE).
        ps_q0 = [
            ppsum.tile([P, FD], F32, tag="pp", bufs=6, name=f"ppq0_{qb}")
            for qb in range(NQB)
        ]
        ps_k0 = [
            ppsum.tile([P, FD], F32, tag="pp", bufs=6, name=f"ppk0_{qb}")
            for qb in range(2)
        ] + [
            p4s.tile([P, FD], F32, tag="p4", bufs=2, name=f"ppk0_{qb}")
            for qb in (2, 3)
        ]
        for c in range(NKC):
            wq0 = strips[(0, "q", c // (NKC // 2))]
            wk0 = strips[(0, "k", c // (NKC // 2))]
            for qb in range(NQB):
                nc.tensor.matmul(
                    ps_q0[qb][:, :],
                    wq0[:, c % (NKC // 2), :],
                    xT[:, c, qb * FD : (qb + 1) * FD],
                    start=(c == 0),
                    stop=(c == NKC - 1),
                )
                nc.tensor.matmul(
                    ps_k0[qb][:, :],
                    wk0[:, c % (NKC // 2), :],
                    xT[:, c, qb * FD : (qb + 1) * FD],
                    start=(c == 0),
                    stop=(c == NKC - 1),
                )
        for qb in range(NQB):
            nc.scalar.activation(
                QT[:, 0, qb * FD : (qb + 1) * FD],
                ps_q0[qb][:, :],
                mybir.ActivationFunctionType.Identity,
                bias=bqk_sb[:, 0, 0:1],
            )
            nc.scalar.activation(
                KT[:, 0, qb * FD : (qb + 1) * FD],
                ps_k0[qb][:, :],
                mybir.ActivationFunctionType.Identity,
                bias=bqk_sb[:, 1, 0:1],
            )

        def v_round(st):
            ps = [
                ppsum.tile([P, FD], F32, tag="pp", bufs=6, name=f"vp{st}_{dc}")
                for dc in range(2)
            ]
            for c in range(NKC):
                for dc in range(2):
                    nc.tensor.matmul(
                        ps[dc][:, :],
                        xT[:, c, st * P : (st + 1) * P],
                        wv_sb[:, c, dc * FD : (dc + 1) * FD],
                        start=(c == 0),
                        stop=(c == NKC - 1),
                    )
            for dc in range(2):
                nc.vector.tensor_copy(V[:, st, dc * FD : (dc + 1) * FD], ps[dc][:, :])

        ri = 0
        vst = 0
        for h in range(1, NHL):
            if 2 <= h + 1 < NHL:
                load_strip(h + 1, nc.gpsimd)
            for kind in ("q", "k"):
                # Deferred-Q heads skip their last q-block here; it is
                # recomputed from a DRAM re-read as qb0 attention filler.
                qbs = range(3) if (kind == "q" and h in DEFQ_HEADS) else range(NQB)
                ps = {
                    qb: ppsum.tile(
                        [P, FD], F32, tag="pp", bufs=6, name=f"pp{kind}{h}_{qb}"
                    )
                    for qb in qbs
                }
                for c in range(NKC):
                    w = strips[(h, kind, c // (NKC // 2))]
                    for qb in qbs:
                        nc.tensor.matmul(
                            ps[qb][:, :],
                            w[:, c % (NKC // 2), :],
                            xT[:, c, qb * FD : (qb + 1) * FD],
                            start=(c == 0),
                            stop=(c == NKC - 1),
                        )
                dst = QT if kind == "q" else KT
                bias = bqk_sb[:, 0 if kind == "q" else 1, h : h + 1]
                for qb in qbs:
                    nc.scalar.activation(
                        dst[:, h, qb * FD : (qb + 1) * FD],
                        ps[qb][:, :],
                        mybir.ActivationFunctionType.Identity,
                        bias=bias,
                    )
                # Interleave one V seq-tile round once x and wv are resident.
                if ri >= 3 and vst < NST:
                    v_round(vst)
                    vst += 1
                ri += 1
        es_strip.close()
        while vst < NST:
            v_round(vst)
            vst += 1
        es_pp.close()
        es_wv.close()
        es_x.close()

        # ------------------------- attention + out-proj -------------------
        # wo goes into the SBUF freed by xT (left stack, after es_x.close()).
        es_attn = ExitStack()
        # x columns of the last q-block + the DEFQ heads' wq strips, re-read
        # from DRAM for the deferred-Q filler (keeping xT/strips resident
        # through attention would not fit SBUF).
        xqpool = es_attn.enter_context(tc.tile_pool(name="xqpool", bufs=1))
        xq3 = xqpool.tile([P, NKC, FD], BF16)
        rstrips = {}
        for h in DEFQ_HEADS:
            for half in range(2):
                rstrips[(h, half)] = xqpool.tile(
                    [P, NKC // 2, P], BF16, name=f"rwq{h}_{half}"
                )
        # Need-order: the first pops touch rstrips[h5] + xq3 chunk 0 first.
        nc.gpsimd.dma_start(out=rstrips[(5, 0)][:, :, :], in_=wq_d[5, 0, :, :, :])
        for i in range(0, NKC, 4):
            nc.gpsimd.dma_start(
                out=xq3[:, i : i + 4, :], in_=xT_d[:, i : i + 4, 3 * FD : S]
            )
            if i == 0:
                nc.gpsimd.dma_start(
                    out=rstrips[(5, 1)][:, :, :], in_=wq_d[5, 1, :, :, :]
                )
        for h in (6, 7):
            for half in range(2):
                nc.gpsimd.dma_start(
                    out=rstrips[(h, half)][:, :, :], in_=wq_d[h, half, :, :, :]
                )
        wopool = es_attn.enter_context(tc.tile_pool(name="wopool", bufs=1))
        wo_sb = wopool.tile([P, DHL // P, DM], BF16)
        for i in range(0, DHL // P, 4):
            nc.gpsimd.dma_start(
                out=wo_sb[:, i : i + 4, :], in_=wo_d[:, i : i + 4, :]
            )

        epool = es_attn.enter_context(tc.tile_pool(name="epool", bufs=5))
        apool = es_attn.enter_context(tc.tile_pool(name="apool", bufs=2))
        rpool = es_attn.enter_context(tc.tile_pool(name="rpool", bufs=2))
        cpool = es_attn.enter_context(tc.tile_pool(name="cpool", bufs=2))
        stpool = es_attn.enter_context(tc.tile_pool(name="stpool", bufs=2))
        # PSUM: sps/pvs/bcs on freed ppsum banks (0-5); p4s on banks 6-7,
        # which no earlier pool ever touched.
        sps = es_attn.enter_context(tc.tile_pool(name="sps", bufs=3, space="PSUM"))
        pvs = es_attn.enter_context(tc.tile_pool(name="pvs", bufs=2, space="PSUM"))
        bcs = es_attn.enter_context(tc.tile_pool(name="bcs", bufs=1, space="PSUM"))

        # Pending out-projection micro-ops with PE-cost and generation
        # tags, popped into attention kt slots by the deficit pacer.
        p4q = deque()  # items: (pe_cost_ns, fn, gen)

        def queue_p4(qb, ctx):
            for stl in range(4):
                st = qb * 4 + stl
                box = {}

                def alloc(box=box, st=st):
                    box["stage"] = stpool.tile(
                        [P, DM], BF16, tag="stage", name=f"stage{st}"
                    )

                p4q.append((0.0, alloc, qb))
                for half in range(2):

                    def mk_ps(box=box, st=st, half=half):
                        box["ps"] = [
                            p4s.tile([P, FD], F32, tag="p4", bufs=2, name=f"o{st}_{half}_{m}")
                            for m in range(2)
                        ]

                    p4q.append((0.0, mk_ps, qb))
                    for dc in range(DHL // P):

                        def mm(box=box, stl=stl, half=half, dc=dc, ctx=ctx):
                            for m in range(2):
                                nc.tensor.matmul(
                                    box["ps"][m][:, :],
                                    ctx[:, dc, stl * P : (stl + 1) * P],
                                    wo_sb[:, dc, (half * 2 + m) * FD : (half * 2 + m + 1) * FD],
                                    start=(dc == 0),
                                    stop=(dc == DHL // P - 1),
                                )

                        p4q.append((MM_COST, mm, qb))

                    def evict_store(box=box, st=st, half=half):
                        for m in range(2):
                            mc = half * 2 + m
                            # PSUM eviction on DVE (GpSimd has no PSUM port;
                            # scalar must stay exp-only during attention but
                            # is idle in the drain, where it halves the final
                            # eviction latency).
                            if st >= 12 and mc % 2 == 1:
                                nc.scalar.copy(
                                    box["stage"][:, mc * FD : (mc + 1) * FD],
                                    box["ps"][m][:, :],
                                )
                            else:
                                nc.vector.tensor_copy(
                                    box["stage"][:, mc * FD : (mc + 1) * FD],
                                    box["ps"][m][:, :],
                                )
                            # Out stores on the HWDGE rings (SWDGE queues stay
                            # empty at kernel end — no long GpSimd drain).
                            # During attention sync-only (scalar = exp pacer);
                            # in the drain phase scalar is idle, so alternate
                            # rings to halve the final-store latency.
                            se = nc.scalar if (st >= 12 and mc % 2 == 1) else nc.sync
                            se.dma_start(
                                out=out_d[st * P : (st + 1) * P, mc * FD : (mc + 1) * FD],
                                in_=box["stage"][:, mc * FD : (mc + 1) * FD],
                            )

                    p4q.append((0.0, evict_store, qb))

        popped_cost = 0.0
        quota = 0.0

        def pop_until(target):
            nonlocal popped_cost
            while p4q and popped_cost < target:
                cost, fn, _ = p4q.popleft()
                fn()
                popped_cost += cost

        def pop_stale(max_gen):
            # ctx(qb) reuses ctx(qb-2)'s buffer (cpool bufs=2): everything
            # queued at generation qb-2 or earlier must be emitted before
            # qb's first ctx write, or the PE pipeline stalls on the reuse.
            nonlocal popped_cost
            while p4q and p4q[0][2] <= max_gen:
                cost, fn, _ = p4q.popleft()
                fn()
                popped_cost += cost

        # Deferred-Q filler: the qb3 Q projection of DEFQ_HEADS, queued ahead
        # of the out-projection so qb0 (which has no p4 backlog yet) has PE
        # work to hide its exp-chain latency behind.
        for h in DEFQ_HEADS:
            qbox = {}

            def qalloc(qbox=qbox, h=h):
                qbox["ps"] = p4s.tile([P, FD], F32, tag="p4", bufs=2, name=f"defq{h}")

            p4q.append((0.0, qalloc, -1))
            for c in range(NKC):

                def qmm(qbox=qbox, h=h, c=c):
                    w = rstrips[(h, c // (NKC // 2))]
                    nc.tensor.matmul(
                        qbox["ps"][:, :],
                        w[:, c % (NKC // 2), :],
                        xq3[:, c, :],
                        start=(c == 0),
                        stop=(c == NKC - 1),
                    )

                p4q.append((213.0, qmm, -1))

            def qevict(qbox=qbox, h=h):
                nc.scalar.activation(
                    QT[:, h, 3 * FD : S],
                    qbox["ps"][:, :],
                    mybir.ActivationFunctionType.Identity,
                    bias=bqk_sb[:, 0, h : h + 1],
                )

            p4q.append((0.0, qevict, -1))

        for qb in range(NQB):
            kmax = 4 * (qb + 1)
            pop_stale(qb - 2)
            ctx = cpool.tile([P, NHL, FD], BF16, tag="ctx", name=f"ctx{qb}")
            tail_a = None
            tail_b = None
            for h in range(NHL):
                acc = apool.tile([P, FD], BF16, tag="acc", name=f"acc{h}_{qb}")
                pv = pvs.tile([P, FD], F32, tag="pv", bufs=2, name=f"pv{h}_{qb}")
                exp_t = {}
                lo_of = {}
                for kt in range(kmax):
                    j = kt - 4 * qb
                    lo = max(j, 0) * P
                    lo_of[kt] = lo
                    sp = sps.tile([P, FD], F32, tag="sps", bufs=3, name=f"s{h}_{qb}_{kt}")
                    nc.tensor.matmul(
                        sp[:, lo:FD],
                        KT[:, h, kt * P : (kt + 1) * P],
                        QT[:, h, qb * FD + lo : (qb + 1) * FD],
                        start=True,
                        stop=True,
                    )
                    if j >= 0:
                        nc.vector.tensor_add(
                            sp[:, lo : lo + P], sp[:, lo : lo + P], umask[:, :]
                        )
                    ex = epool.tile([P, FD], BF16, tag="exp", name=f"e{h}_{qb}_{kt}")
                    nc.scalar.activation(
                        ex[:, lo:FD],
                        sp[:, lo:FD],
                        mybir.ActivationFunctionType.Exp,
                        scale=SCALE,
                    )
                    # Row-sum accumulation on DVE (GpSimd's software tensor
                    # ops are ~5x slower and serialize the per-head chain).
                    if kt == 0:
                        nc.vector.tensor_copy(acc[:, :], ex[:, :])
                    else:
                        nc.vector.tensor_add(
                            acc[:, lo:FD], acc[:, lo:FD], ex[:, lo:FD]
                        )
                    exp_t[kt] = ex
                    if kt > 0:
                        pkt = kt - 1
                        plo = lo_of[pkt]
                        nc.tensor.matmul(
                            pv[:, plo:FD],
                            V[:, pkt, h * P : (h + 1) * P],
                            exp_t[pkt][:, plo:FD],
                            start=(pkt == 0),
                            stop=False,
                        )
                    # Deficit-paced out-projection filler for qb-1. Head 0 of
                    # qb0 pops nothing: the deferred-Q re-reads (xq3/rstrips)
                    # may still be in flight right at attention start.
                    if not (qb == 0 and h <= 1):
                        quota += DEFICIT_NS[qb] / kmax
                    pop_until(quota)
                    # Previous head's tail lands here, split: the final PV
                    # (scalar-exp dependent) four slots in, the normalizer
                    # (DVE acc-chain dependent) six slots in.
                    if kt == min(3, kmax - 1) and tail_a is not None:
                        tail_a()
                        tail_a = None
                    if kt == min(5, kmax - 1) and tail_b is not None:
                        tail_b()
                        tail_b = None

                def mk_tails(
                    h=h,
                    qb=qb,
                    kmax=kmax,
                    acc=acc,
                    pv=pv,
                    ex=exp_t[kmax - 1],
                    plo=lo_of[kmax - 1],
                    ctx=ctx,
                ):
                    def ta():
                        nc.tensor.matmul(
                            pv[:, plo:FD],
                            V[:, kmax - 1, h * P : (h + 1) * P],
                            ex[:, plo:FD],
                            start=(kmax == 1),
                            stop=True,
                        )

                    def tb():
                        # Normalizer: partition-reduce + broadcast in one
                        # matmul.
                        bc = bcs.tile([P, FD], F32, tag="bc", bufs=1, name=f"bc{h}_{qb}")
                        nc.tensor.matmul(
                            bc[:, :], ones_sb[:, :], acc[:, :], start=True, stop=True
                        )
                        recip = rpool.tile([P, FD], F32, tag="recip", name=f"r{h}_{qb}")
                        nc.vector.reciprocal_approx_fast(out=recip[:, :], in_=bc[:, :])
                        # Normalize ctx straight from the pv PSUM bank (saves
                        # a [P,512] f32 DVE copy per head; the bank is freed
                        # here, still two heads before its next writer).
                        nc.vector.tensor_mul(ctx[:, h, :], pv[:, :], recip[:, :])

                    return ta, tb

                tail_a, tail_b = mk_tails()
            # Last head of the q-block: cover the exp latency with forced
            # pops (when backlog exists), then flush.
            quota = max(quota, popped_cost + 2 * MM_COST)
            pop_until(quota)
            tail_a()
            tail_b()
            tail_a = None
            tail_b = None
            queue_p4(qb, ctx)
        while p4q:
            cost, fn, _ = p4q.popleft()
            fn()
        es_attn.close()
        es_v.close()
        es_main.close()

    # Populate .instr bytes for the custom-DVE InstISA (reciprocal_approx) —
    # raw Bass skips this Bacc pass and the NEFF compiler rejects the empty
    # encoding with "ISA wrong length".
    mybir.codegen_inst_isa_subclasses(nc)
    if fix_waits:
        _fix_sync_waits(nc)
    return nc


def shard_inputs(x, Wq, bq, Wk, bk, Wv, bv, Wo, bo):
    """Host-side sharding: returns per-core input maps (bf16 pre-arranged)."""
    xTs = []
    for b in range(B):
        xt = np.ascontiguousarray(np.asarray(x)[b].T).astype(NP_BF16)  # [dm, seq]
        xTs.append(np.ascontiguousarray(xt.reshape(NKC, P, S).transpose(1, 0, 2)))
    wqs, wks, wvs, wos, bqks = [], [], [], [], []
    for g in range(G):
        sl = slice(g * DHL, (g + 1) * DHL)
        wq_s = np.asarray(Wq)[:, sl].astype(NP_BF16)
        wk_s = np.asarray(Wk)[:, sl].astype(NP_BF16)
        wv_s = np.asarray(Wv)[:, sl].astype(NP_BF16)
        wo_s = np.asarray(Wo)[sl, :].astype(NP_BF16)
        wqs.append(
            np.ascontiguousarray(
                wq_s.reshape(2, NKC // 2, P, NHL, P).transpose(3, 0, 2, 1, 4)
            )
        )
        wks.append(
            np.ascontiguousarray(
                wk_s.reshape(2, NKC // 2, P, NHL, P).transpose(3, 0, 2, 1, 4)
            )
        )
        wvs.append(np.ascontiguousarray(wv_s.reshape(NKC, P, DHL).transpose(1, 0, 2)))
        wos.append(
            np.ascontiguousarray(wo_s.reshape(DHL // P, P, DM).transpose(1, 0, 2))
        )
        bqk = np.stack(
            [
                np.asarray(bq, np.float32)[sl].reshape(NHL, P),
                np.asarray(bk, np.float32)[sl].reshape(NHL, P),
            ]
        )  # [2, nhl, P]
        bqks.append(np.ascontiguousarray(bqk.transpose(2, 0, 1)))  # [P, 2, nhl]
    in_maps = []
    for c in range(B * G):
        b, g = divmod(c, G)
        in_maps.append(
            {
                "xT": xTs[b],
                "wq": wqs[g],
                "wk": wks[g],
                "wv": wvs[g],
                "wo": wos[g],
                "bqk": bqks[g],
            }
        )
    return in_maps


_CACHE = {}


def _get_nc():
    if "nc" not in _CACHE:
        _CACHE["nc"] = build_nc()
    return _CACHE["nc"]


def run(inputs, trace=False):
    """Run the SPMD kernel; returns (full_output, BassKernelResults)."""
    inputs = {k: np.asarray(v) for k, v in inputs.items()}
    nc = _get_nc()
    in_maps = shard_inputs(**inputs)
    res = run_bass_kernel_spmd(
        nc, in_maps, core_ids=list(range(NCORES)), trace=trace
    )
    Wo = np.asarray(inputs["Wo"], np.float32)
    const_row = (
        np.asarray(inputs["bv"], np.float32) @ Wo + np.asarray(inputs["bo"], np.float32)
    )
    out = np.empty((B, S, DM), np.float32)
    for b in range(B):
        out[b] = (
            res.results[G * b]["out"].astype(np.float32)
            + res.results[G * b + 1]["out"].astype(np.float32)
            + const_row
        )
    return out, res


def kernel(**inputs):
    out, _ = run(inputs, trace=False)
    return out


# revision 53
# speedup vs baseline: 1.0043x; 1.0043x over previous
"""Causal self-attention Trainium2 kernel (8 NeuronCores, SPMD) — v3.

Sharding: 8 cores = 4 batches x 2 head-groups. Each core computes, for its
(batch b, head-group g): Q/K/V projections restricted to g's 8 heads
(column-parallel), causal attention for those heads, and the partial output
projection ctx_g @ Wo[g rows] (row-parallel). Host sums the two partials per
batch and adds the bias terms (bv @ Wo + bo).

v3 changes vs v2 (v2 measured 634 us, PE busy 91%, ~58 us PE idle; v3
measures ~627.5 us, P1/V phase fully PE-dense at 99-100%):
- P1/P1b fusion: Q/K rounds and V seq-tile rounds share one 6-bank PSUM pool
  with V rounds interleaved between Q/K rounds, so there is no P1->P1b
  transition stall. Head 0 runs a special 8-bank q+k-interleaved round
  (borrowing the p4s banks, idle during P1) so its chunk consumption
  (1.7us/chunk) stays above the x DMA delivery rate (~1.4us/chunk) — the PE
  is work-limited from the first matmul on.
- DMA head: x moves as 16 full chunks on SWDGE (issue-rate bound), first
  strips by need-order; deadlock-aware strip ordering (h1 strips after all x
  issues: they reuse h0's slots and would block the gpsimd queue).
- Head-tail software pipelining: each head's closing ops are deferred into
  the NEXT head's kt loop, split in two (final PV + pv copy at kt==3;
  normalizer matmul + reciprocal + ctx mul at kt==5), so the PE never sits
  on the score->exp->acc dependency chains at head boundaries.
- Deficit-paced out-projection interleave: scalar exp is the local pacer in
  later q-blocks; p4 pops are paced by the per-q-block scalar-PE deficit.
  The qb3 Q projection of heads 5-7 is deferred out of P1 (re-reading x cols
  and wq strips from DRAM) and injected as filler for qb0, which otherwise
  has no backlog.
- Engine placement tuned by trace: exp + Q/K bias on scalar only, PSUM
  evictions and all normalization on DVE (cross-engine offloads of
  chain-adjacent ops to gpsimd measurably backfire on semaphore latency),
  out stores on the sync HWDGE ring (scalar helps only in the drain), SWDGE
  queues empty at kernel end (no long GpSimd drain).
- Output is written in bf16 (host upcasts and sums the two group partials).
"""

import sys

sys.path.insert(0, "/opt/trn_rl_repo")

from collections import deque
from contextlib import ExitStack

import numpy as np

import concourse.bass as bass
import concourse.tile as tile
from concourse import mybir
from concourse.bass_utils import run_bass_kernel_spmd

BF16 = mybir.dt.bfloat16
F32 = mybir.dt.float32
NP_BF16 = mybir.dt.np(BF16)

# Problem constants (hardcoded per contract).
B = 4          # batch
S = 2048       # sequence length
DM = 2048      # d_model
H = 16         # total heads
HD = 128       # head dim
G = 2          # head groups (tensor parallel degree)
NHL = H // G   # local heads per core
DHL = NHL * HD # local head dims
NCORES = 8
P = 128        # partitions
FD = 512       # matmul moving free dim (one PSUM bank of f32)
NKC = DM // P  # contraction chunks for projections
NST = S // P   # seq tiles (k tiles)
NQB = S // FD  # 512-wide q blocks
SCALE = 1.0 / float(np.sqrt(HD))
MASK_VAL = -1e30

# Per-head scalar-vs-PE deficit (ns) per q-block: how much PE filler each
# head's attention needs so the PE does not outrun the exp stream and stall
# on the score->exp->PV chain. Statically derived from measured ACTIVATE
# cost (259 ns + 0.836 ns/col) vs matmul cost (213 ns / 512 cols).
DEFICIT_NS = {0: 2100.0, 1: 2500.0, 2: 3400.0, 3: 5000.0}
MM_COST = 426.0  # one p4 micro-op = 2 matmuls of 512 rows
# Heads whose Q projection for the last q-block is deferred out of P1 and
# injected as PE filler during qb0 (which otherwise has no pop backlog).
DEFQ_HEADS = (5, 6, 7)

_WAIT_EXEMPT = {
    "NoOp",
    "EventSemaphore",
    "UnconditionalBranch",
    "RegisterMove",
    "TileRelease",
}


def _fix_sync_waits(nc, max_waits=1):
    """Hoist extra sync-waits onto single-wait NoOps on the issuing engine.

    Several walrus instruction encodings (PSEUDO_DMA_DIRECT2D, S3_LW, CTRL_NO,
    ...) have a single sync-wait slot and fail codegen with "Too many sync
    wait commands" when Tile attaches more. A NoOp on the same engine
    immediately before the instruction performs the extra wait at the
    sequencer, which is semantically identical.
    """
    f = nc.m.functions[0]
    fixed = 0

    def walk(blocks):
        nonlocal fixed
        for b in blocks:
            il = b.instructions
            i = 0
            while i < len(il):
                inst = il[i]
                si = getattr(inst, "sync_info", None)
                ow = list(si.on_wait) if si is not None and si.on_wait else []
                if inst.opcode not in _WAIT_EXEMPT and len(ow) > max_waits:
                    keep = ow[len(ow) - max_waits :]
                    extra = ow[: len(ow) - max_waits]
                    for j, w in enumerate(extra):
                        nop = mybir.InstNoOp(
                            name=f"{inst.name}_waitfix{j}",
                            engine=inst.engine,
                            ins=[],
                            outs=[],
                            bass_nofuse=True,
                            sync_info=mybir.SyncInfo(on_wait=[w], on_update=[]),
                        )
                        il.insert(i, nop)
                        i += 1
                    inst.sync_info = mybir.SyncInfo(
                        on_wait=keep,
                        on_update=list(si.on_update) if si.on_update else [],
                    )
                    fixed += 1
                i += 1
            walk(getattr(b, "blocks", []) or [])

    walk(f.blocks)
    return fixed


def build_nc(fix_waits=True):
    """Build the single-core Bass program (same program for all 8 cores)."""
    nc = bass.Bass()
    # Inputs are pre-arranged on the host so every DMA line is contiguous.
    # wq/wk are half-strip-major so one [P, NKC//2, P] half is a contiguous
    # 2 KB line per partition (256 B lines are below SDMA line rate).
    xT_d = nc.dram_tensor("xT", [P, NKC, S], BF16, kind="ExternalInput")
    wq_d = nc.dram_tensor("wq", [NHL, 2, P, NKC // 2, P], BF16, kind="ExternalInput")
    wk_d = nc.dram_tensor("wk", [NHL, 2, P, NKC // 2, P], BF16, kind="ExternalInput")
    wv_d = nc.dram_tensor("wv", [P, NKC, DHL], BF16, kind="ExternalInput")
    wo_d = nc.dram_tensor("wo", [P, DHL // P, DM], BF16, kind="ExternalInput")
    bqk_d = nc.dram_tensor("bqk", [P, 2, NHL], F32, kind="ExternalInput")
    out_d = nc.dram_tensor("out", [S, DM], BF16, kind="ExternalOutput")

    with tile.TileContext(nc) as tc:
        # ------------------------- pools (left stack) ---------------------
        es_main = ExitStack()
        consts = es_main.enter_context(tc.tile_pool(name="consts", bufs=1))
        bqk_sb = consts.tile([P, 2, NHL], F32)
        ones_sb = consts.tile([P, P], BF16)
        umask = consts.tile([P, P], F32)

        qkv = es_main.enter_context(tc.tile_pool(name="qkv", bufs=1))
        QT = qkv.tile([P, NHL, S], BF16)
        KT = qkv.tile([P, NHL, S], BF16)

        es_x = ExitStack()
        xpool = es_x.enter_context(tc.tile_pool(name="xpool", bufs=1))
        xT = xpool.tile([P, NKC, S], BF16)

        # ------------------------- pools (right stack) --------------------
        # LIFO close order: strips (end P1) -> wv (end P1b) -> V (end).
        es_v = ExitStack()
        vpool = es_v.enter_context(tc.tile_pool(name="vpool", bufs=1, side="right"))
        V = vpool.tile([P, NST, DHL], BF16)

        es_wv = ExitStack()
        wvpool = es_wv.enter_context(
            tc.tile_pool(name="wvpool", bufs=1, side="right")
        )
        wv_sb = wvpool.tile([P, NKC, DHL], BF16)

        es_strip = ExitStack()
        spool = es_strip.enter_context(
            tc.tile_pool(name="spool", bufs=6, side="right")
        )

        # ------------------------- DMA issue (order = priority) -----------
        # The SWDGE (gpsimd) ring spreads consecutive dma_starts across ~16
        # parallel queue rows. SWDGE issue costs ~0.6us per dma_start; x goes
        # first, split per (chunk, seq-half): 32 issues. The first q/k strips
        # ride the two HWDGE rings (issued by the otherwise idle sync and
        # scalar engines) so the first P1 matmul starts at ~2us.
        strips = {}  # (h, 'q'|'k', half) -> tile

        def load_strip(h, eng):
            for kind, src in (("q", wq_d), ("k", wk_d)):
                for half in range(2):
                    t = spool.tile(
                        [P, NKC // 2, P], BF16, tag="strip", name=f"w{kind}{h}_{half}"
                    )
                    eng.dma_start(out=t[:, :, :], in_=src[h, half, :, :, :])
                    strips[(h, kind, half)] = t

        nc.sync.dma_start(out=bqk_sb[:, :, :], in_=bqk_d[:, :, :])
        # Everything on SWDGE (per-transfer packets spread over all 16 DMA
        # engines; the aggregate ~360 GB/s HBM rate is the wall, so order
        # strictly by need). x moves as FULL chunks (4 KB lines, one issue
        # each). P1's h0 interleaves q AND k per chunk (1.7 us/chunk work vs
        # ~1.4 us/chunk delivery), so the PE is work-limited from the first
        # matmul on.
        def strip_piece(h, kind, half, eng):
            src = wq_d if kind == "q" else wk_d
            t = spool.tile(
                [P, NKC // 2, P], BF16, tag="strip", name=f"w{kind}{h}_{half}"
            )
            eng.dma_start(out=t[:, :, :], in_=src[h, half, :, :, :])
            strips[(h, kind, half)] = t

        nc.gpsimd.dma_start(out=xT[:, 0, :], in_=xT_d[:, 0, :])
        strip_piece(0, "q", 0, nc.gpsimd)
        strip_piece(0, "k", 0, nc.gpsimd)
        for i in range(1, NKC):
            nc.gpsimd.dma_start(out=xT[:, i, :], in_=xT_d[:, i, :])
            if i == 4:
                strip_piece(0, "q", 1, nc.gpsimd)
                strip_piece(0, "k", 1, nc.gpsimd)
        # h1 strips AFTER all x issues: their DMAs reuse h0's strip slots and
        # wait on h0 consumption — anything queued behind them on the gpsimd
        # engine would deadlock against P1's x needs.
        load_strip(1, nc.gpsimd)
        for j in range(0, NKC, 4):
            nc.gpsimd.dma_start(
                out=wv_sb[:, j : j + 4, :], in_=wv_d[:, j : j + 4, :]
            )

        # ------------------------- constants setup ------------------------
        nc.vector.memset(ones_sb[:, :], 1.0)
        # umask[k, q] = 0 if q >= k else MASK_VAL (transposed diagonal block).
        nc.gpsimd.memset(umask[:, :], 0.0)
        nc.gpsimd.affine_select(
            out=umask[:, :],
            in_=umask[:, :],
            compare_op=mybir.AluOpType.is_ge,
            fill=MASK_VAL,
            base=0,
            pattern=[[1, P]],
            channel_multiplier=-1,
        )

        # ------------------------- P1 + P1b: projections -------------------
        # One shared 6-bank PSUM pool for Q/K rounds (4 tiles) and V rounds
        # (2 tiles): V seq-tile rounds are interleaved between Q/K rounds
        # (one per round once x/wv are resident), so there is no P1->P1b
        # transition stall. p4s (banks 6-7, program-lifetime, right stack) is
        # idle during P1 and lends its 2 banks to h0's 8-bank q+k round.
        es_pp = ExitStack()
        ppsum = es_pp.enter_context(tc.tile_pool(name="ppsum", bufs=6, space="PSUM"))
        p4s = es_main.enter_context(
            tc.tile_pool(name="p4s", bufs=2, space="PSUM", side="right")
        )

        # h0: q and k interleaved per chunk, paced to x chunk arrival (the
        # x stream is still in flight; a q-only round would starve the PE).
        ps_q0 = [
            ppsum.tile([P, FD], F32, tag="pp", bufs=6, name=f"ppq0_{qb}")
            for qb in range(NQB)
        ]
        ps_k0 = [
            ppsum.tile([P, FD], F32, tag="pp", bufs=6, name=f"ppk0_{qb}")
            for qb in range(2)
        ] + [
            p4s.tile([P, FD], F32, tag="p4", bufs=2, name=f"ppk0_{qb}")
            for qb in (2, 3)
        ]
        for c in range(NKC):
            wq0 = strips[(0, "q", c // (NKC // 2))]
            wk0 = strips[(0, "k", c // (NKC // 2))]
            for qb in range(NQB):
                nc.tensor.matmul(
                    ps_q0[qb][:, :],
                    wq0[:, c % (NKC // 2), :],
                    xT[:, c, qb * FD : (qb + 1) * FD],
                    start=(c == 0),
                    stop=(c == NKC - 1),
                )
                nc.tensor.matmul(
                    ps_k0[qb][:, :],
                    wk0[:, c % (NKC // 2), :],
                    xT[:, c, qb * FD : (qb + 1) * FD],
                    start=(c == 0),
                    stop=(c == NKC - 1),
                )
        for qb in range(NQB):
            nc.scalar.activation(
                QT[:, 0, qb * FD : (qb + 1) * FD],
                ps_q0[qb][:, :],
                mybir.ActivationFunctionType.Identity,
                bias=bqk_sb[:, 0, 0:1],
            )
            nc.scalar.activation(
                KT[:, 0, qb * FD : (qb + 1) * FD],
                ps_k0[qb][:, :],
                mybir.ActivationFunctionType.Identity,
                bias=bqk_sb[:, 1, 0:1],
            )

        def v_round(st):
            ps = [
                ppsum.tile([P, FD], F32, tag="pp", bufs=6, name=f"vp{st}_{dc}")
                for dc in range(2)
            ]
            for c in range(NKC):
                for dc in range(2):
                    nc.tensor.matmul(
                        ps[dc][:, :],
                        xT[:, c, st * P : (st + 1) * P],
                        wv_sb[:, c, dc * FD : (dc + 1) * FD],
                        start=(c == 0),
                        stop=(c == NKC - 1),
                    )
            for dc in range(2):
                nc.vector.tensor_copy(V[:, st, dc * FD : (dc + 1) * FD], ps[dc][:, :])

        ri = 0
        vst = 0
        for h in range(1, NHL):
            if 2 <= h + 1 < NHL:
                load_strip(h + 1, nc.gpsimd)
            for kind in ("q", "k"):
                # Deferred-Q heads skip their last q-block here; it is
                # recomputed from a DRAM re-read as qb0 attention filler.
                qbs = range(3) if (kind == "q" and h in DEFQ_HEADS) else range(NQB)
                ps = {
                    qb: ppsum.tile(
                        [P, FD], F32, tag="pp", bufs=6, name=f"pp{kind}{h}_{qb}"
                    )
                    for qb in qbs
                }
                for c in range(NKC):
                    w = strips[(h, kind, c // (NKC // 2))]
                    for qb in qbs:
                        nc.tensor.matmul(
                            ps[qb][:, :],
                            w[:, c % (NKC // 2), :],
                            xT[:, c, qb * FD : (qb + 1) * FD],
                            start=(c == 0),
                            stop=(c == NKC - 1),
                        )
                dst = QT if kind == "q" else KT
                bias = bqk_sb[:, 0 if kind == "q" else 1, h : h + 1]
                for qb in qbs:
                    nc.scalar.activation(
                        dst[:, h, qb * FD : (qb + 1) * FD],
                        ps[qb][:, :],
                        mybir.ActivationFunctionType.Identity,
                        bias=bias,
                    )
                # Interleave one V seq-tile round once x and wv are resident.
                if ri >= 3 and vst < NST:
                    v_round(vst)
                    vst += 1
                ri += 1
        es_strip.close()
        while vst < NST:
            v_round(vst)
            vst += 1
        es_pp.close()
        es_wv.close()
        es_x.close()

        # ------------------------- attention + out-proj -------------------
        # wo goes into the SBUF freed by xT (left stack, after es_x.close()).
        es_attn = ExitStack()
        # x columns of the last q-block + the DEFQ heads' wq strips, re-read
        # from DRAM for the deferred-Q filler (keeping xT/strips resident
        # through attention would not fit SBUF).
        xqpool = es_attn.enter_context(tc.tile_pool(name="xqpool", bufs=1))
        xq3 = xqpool.tile([P, NKC, FD], BF16)
        rstrips = {}
        for h in DEFQ_HEADS:
            for half in range(2):
                rstrips[(h, half)] = xqpool.tile(
                    [P, NKC // 2, P], BF16, name=f"rwq{h}_{half}"
                )
        # Need-order: the first pops touch rstrips[h5] + xq3 chunk 0 first.
        nc.gpsimd.dma_start(out=rstrips[(5, 0)][:, :, :], in_=wq_d[5, 0, :, :, :])
        for i in range(0, NKC, 4):
            nc.gpsimd.dma_start(
                out=xq3[:, i : i + 4, :], in_=xT_d[:, i : i + 4, 3 * FD : S]
            )
            if i == 0:
                nc.gpsimd.dma_start(
                    out=rstrips[(5, 1)][:, :, :], in_=wq_d[5, 1, :, :, :]
                )
        for h in (6, 7):
            for half in range(2):
                nc.gpsimd.dma_start(
                    out=rstrips[(h, half)][:, :, :], in_=wq_d[h, half, :, :, :]
                )
        wopool = es_attn.enter_context(tc.tile_pool(name="wopool", bufs=1))
        wo_sb = wopool.tile([P, DHL // P, DM], BF16)
        for i in range(0, DHL // P, 4):
            nc.gpsimd.dma_start(
                out=wo_sb[:, i : i + 4, :], in_=wo_d[:, i : i + 4, :]
            )

        epool = es_attn.enter_context(tc.tile_pool(name="epool", bufs=5))
        apool = es_attn.enter_context(tc.tile_pool(name="apool", bufs=2))
        rpool = es_attn.enter_context(tc.tile_pool(name="rpool", bufs=2))
        cpool = es_attn.enter_context(tc.tile_pool(name="cpool", bufs=2))
        stpool = es_attn.enter_context(tc.tile_pool(name="stpool", bufs=2))
        # PSUM: sps/pvs/bcs on freed ppsum banks (0-5); p4s on banks 6-7,
        # which no earlier pool ever touched.
        sps = es_attn.enter_context(tc.tile_pool(name="sps", bufs=3, space="PSUM"))
        pvs = es_attn.enter_context(tc.tile_pool(name="pvs", bufs=2, space="PSUM"))
        bcs = es_attn.enter_context(tc.tile_pool(name="bcs", bufs=1, space="PSUM"))

        # Pending out-projection micro-ops with PE-cost and generation
        # tags, popped into attention kt slots by the deficit pacer.
        p4q = deque()  # items: (pe_cost_ns, fn, gen)

        def queue_p4(qb, ctx):
            for stl in range(4):
                st = qb * 4 + stl
                box = {}

                def alloc(box=box, st=st):
                    box["stage"] = stpool.tile(
                        [P, DM], BF16, tag="stage", name=f"stage{st}"
                    )

                p4q.append((0.0, alloc, qb))
                for half in range(2):

                    def mk_ps(box=box, st=st, half=half):
                        box["ps"] = [
                            p4s.tile([P, FD], F32, tag="p4", bufs=2, name=f"o{st}_{half}_{m}")
                            for m in range(2)
                        ]

                    p4q.append((0.0, mk_ps, qb))
                    for dc in range(DHL // P):

                        def mm(box=box, stl=stl, half=half, dc=dc, ctx=ctx):
                            for m in range(2):
                                nc.tensor.matmul(
                                    box["ps"][m][:, :],
                                    ctx[:, dc, stl * P : (stl + 1) * P],
                                    wo_sb[:, dc, (half * 2 + m) * FD : (half * 2 + m + 1) * FD],
                                    start=(dc == 0),
                                    stop=(dc == DHL // P - 1),
                                )

                        p4q.append((MM_COST, mm, qb))

                    def evict_store(box=box, st=st, half=half):
                        for m in range(2):
                            mc = half * 2 + m
                            # PSUM eviction on DVE (GpSimd has no PSUM port;
                            # scalar must stay exp-only during attention but
                            # is idle in the drain, where it halves the final
                            # eviction latency).
                            if st >= 12 and mc % 2 == 1:
                                nc.scalar.copy(
                                    box["stage"][:, mc * FD : (mc + 1) * FD],
                                    box["ps"][m][:, :],
                                )
                            else:
                                nc.vector.tensor_copy(
                                    box["stage"][:, mc * FD : (mc + 1) * FD],
                                    box["ps"][m][:, :],
                                )
                            # Out stores on the HWDGE rings (SWDGE queues stay
                            # empty at kernel end — no long GpSimd drain).
                            # During attention sync-only (scalar = exp pacer);
                            # in the drain phase scalar is idle, so alternate
                            # rings to halve the final-store latency.
                            se = nc.scalar if (st >= 12 and mc % 2 == 1) else nc.sync
                            se.dma_start(
                                out=out_d[st * P : (st + 1) * P, mc * FD : (mc + 1) * FD],
                                in_=box["stage"][:, mc * FD : (mc + 1) * FD],
                            )

                    p4q.append((0.0, evict_store, qb))

        popped_cost = 0.0
        quota = 0.0

        def pop_until(target):
            nonlocal popped_cost
            while p4q and popped_cost < target:
                cost, fn, _ = p4q.popleft()
                fn()
                popped_cost += cost

        def pop_stale(max_gen):
            # ctx(qb) reuses ctx(qb-2)'s buffer (cpool bufs=2): everything
            # queued at generation qb-2 or earlier must be emitted before
            # qb's first ctx write, or the PE pipeline stalls on the reuse.
            nonlocal popped_cost
            while p4q and p4q[0][2] <= max_gen:
                cost, fn, _ = p4q.popleft()
                fn()
                popped_cost += cost

        # Deferred-Q filler: the qb3 Q projection of DEFQ_HEADS, queued ahead
        # of the out-projection so qb0 (which has no p4 backlog yet) has PE
        # work to hide its exp-chain latency behind.
        for h in DEFQ_HEADS:
            qbox = {}

            def qalloc(qbox=qbox, h=h):
                qbox["ps"] = p4s.tile([P, FD], F32, tag="p4", bufs=2, name=f"defq{h}")

            p4q.append((0.0, qalloc, -1))
            for c in range(NKC):

                def qmm(qbox=qbox, h=h, c=c):
                    w = rstrips[(h, c // (NKC // 2))]
                    nc.tensor.matmul(
                        qbox["ps"][:, :],
                        w[:, c % (NKC // 2), :],
                        xq3[:, c, :],
                        start=(c == 0),
                        stop=(c == NKC - 1),
                    )

                p4q.append((213.0, qmm, -1))

            def qevict(qbox=qbox, h=h):
                nc.scalar.activation(
                    QT[:, h, 3 * FD : S],
                    qbox["ps"][:, :],
                    mybir.ActivationFunctionType.Identity,
                    bias=bqk_sb[:, 0, h : h + 1],
                )

            p4q.append((0.0, qevict, -1))

        for qb in range(NQB):
            kmax = 4 * (qb + 1)
            pop_stale(qb - 2)
            ctx = cpool.tile([P, NHL, FD], BF16, tag="ctx", name=f"ctx{qb}")
            tail_a = None
            tail_b = None
            for h in range(NHL):
                acc = apool.tile([P, FD], BF16, tag="acc", name=f"acc{h}_{qb}")
                pv = pvs.tile([P, FD], F32, tag="pv", bufs=2, name=f"pv{h}_{qb}")
                exp_t = {}
                lo_of = {}
                for kt in range(kmax):
                    j = kt - 4 * qb
                    lo = max(j, 0) * P
                    lo_of[kt] = lo
                    sp = sps.tile([P, FD], F32, tag="sps", bufs=3, name=f"s{h}_{qb}_{kt}")
                    nc.tensor.matmul(
                        sp[:, lo:FD],
                        KT[:, h, kt * P : (kt + 1) * P],
                        QT[:, h, qb * FD + lo : (qb + 1) * FD],
                        start=True,
                        stop=True,
                    )
                    if j >= 0:
                        nc.vector.tensor_add(
                            sp[:, lo : lo + P], sp[:, lo : lo + P], umask[:, :]
                        )
                    ex = epool.tile([P, FD], BF16, tag="exp", name=f"e{h}_{qb}_{kt}")
                    nc.scalar.activation(
                        ex[:, lo:FD],
                        sp[:, lo:FD],
                        mybir.ActivationFunctionType.Exp,
                        scale=SCALE,
                    )
                    # Row-sum accumulation on DVE (GpSimd's software tensor
                    # ops are ~5x slower and serialize the per-head chain).
                    if kt == 0:
                        nc.vector.tensor_copy(acc[:, :], ex[:, :])
                    else:
                        nc.vector.tensor_add(
                            acc[:, lo:FD], acc[:, lo:FD], ex[:, lo:FD]
                        )
                    exp_t[kt] = ex
                    if kt > 0:
                        pkt = kt - 1
                        plo = lo_of[pkt]
                        nc.tensor.matmul(
                            pv[:, plo:FD],
                            V[:, pkt, h * P : (h + 1) * P],
                            exp_t[pkt][:, plo:FD],
                            start=(pkt == 0),
                            stop=False,
                        )
                    # Deficit-paced out-projection filler for qb-1. Head 0 of
                    # qb0 pops nothing: the deferred-Q re-reads (xq3/rstrips)
                    # may still be in flight right at attention start.
                    if not (qb == 0 and h <= 1):
                        quota += DEFICIT_NS[qb] / kmax
                    pop_until(quota)
                    # Previous head's tail lands here, split: the final PV
                    # (scalar-exp dependent) four slots in, the normalizer
                    # (DVE acc-chain dependent) six slots in.
                    if kt == min(3, kmax - 1) and tail_a is not None:
                        tail_a()
                        tail_a = None
                    if kt == min(5, kmax - 1) and tail_b is not None:
                        tail_b()
                        tail_b = None

                def mk_tails(
                    h=h,
                    qb=qb,
                    kmax=kmax,
                    acc=acc,
                    pv=pv,
                    ex=exp_t[kmax - 1],
                    plo=lo_of[kmax - 1],
                    ctx=ctx,
                ):
                    def ta():
                        nc.tensor.matmul(
                            pv[:, plo:FD],
                            V[:, kmax - 1, h * P : (h + 1) * P],
                            ex[:, plo:FD],
                            start=(kmax == 1),
                            stop=True,
                        )

                    def tb():
                        # Normalizer: partition-reduce + broadcast in one
                        # matmul.
                        bc = bcs.tile([P, FD], F32, tag="bc", bufs=1, name=f"bc{h}_{qb}")
                        nc.tensor.matmul(
                            bc[:, :], ones_sb[:, :], acc[:, :], start=True, stop=True
                        )
                        recip = rpool.tile([P, FD], F32, tag="recip", name=f"r{h}_{qb}")
                        nc.vector.reciprocal_approx_fast(out=recip[:, :], in_=bc[:, :])
                        # Normalize ctx straight from the pv PSUM bank (saves
                        # a [P,512] f32 DVE copy per head; the bank is freed
                        # here, still two heads before its next writer).
                        nc.vector.tensor_mul(ctx[:, h, :], pv[:, :], recip[:, :])

                    return ta, tb

                tail_a, tail_b = mk_tails()
            # Last head of the q-block: cover the exp latency with forced
            # pops (when backlog exists), then flush.
            quota = max(quota, popped_cost + 2 * MM_COST)
            pop_until(quota)
            tail_a()
            tail_b()
            tail_a = None
            tail_b = None
            queue_p4(qb, ctx)
        while p4q:
            cost, fn, _ = p4q.popleft()
            fn()
        es_attn.close()
        es_v.close()
        es_main.close()

    # Populate .instr bytes for the custom-DVE InstISA (reciprocal_approx) —
    # raw Bass skips this Bacc pass and the NEFF compiler rejects the empty
    # encoding with "ISA wrong length".
    mybir.codegen_inst_isa_subclasses(nc)
    if fix_waits:
        _fix_sync_waits(nc)
    return nc


def shard_inputs(x, Wq, bq, Wk, bk, Wv, bv, Wo, bo):
    """Host-side sharding: returns per-core input maps (bf16 pre-arranged)."""
    xTs = []
    for b in range(B):
        xt = np.ascontiguousarray(np.asarray(x)[b].T).astype(NP_BF16)  # [dm, seq]
        xTs.append(np.ascontiguousarray(xt.reshape(NKC, P, S).transpose(1, 0, 2)))
    wqs, wks, wvs, wos, bqks = [], [], [], [], []
    for g in range(G):
        sl = slice(g * DHL, (g + 1) * DHL)
        wq_s = np.asarray(Wq)[:, sl].astype(NP_BF16)
        wk_s = np.asarray(Wk)[:, sl].astype(NP_BF16)
        wv_s = np.asarray(Wv)[:, sl].astype(NP_BF16)
        wo_s = np.asarray(Wo)[sl, :].astype(NP_BF16)
        wqs.append(
            np.ascontiguousarray(
                wq_s.reshape(2, NKC // 2, P, NHL, P).transpose(3, 0, 2, 1, 4)
            )
        )
        wks.append(
            np.ascontiguousarray(
                wk_s.reshape(2, NKC // 2, P, NHL, P).transpose(3, 0, 2, 1, 4)
            )
        )
        wvs.append(np.ascontiguousarray(wv_s.reshape(NKC, P, DHL).transpose(1, 0, 2)))
        wos.append(
            np.ascontiguousarray(wo_s.reshape(DHL // P, P, DM).transpose(1, 0, 2))
        )
        bqk = np.stack(
            [
                np.asarray(bq, np.float32)[sl].reshape(NHL, P),
                np.asarray(bk, np.float32)[sl].reshape(NHL, P),
            ]
        )  # [2, nhl, P]
        bqks.append(np.ascontiguousarray(bqk.transpose(2, 0, 1)))  # [P, 2, nhl]
    in_maps = []
    for c in range(B * G):
        b, g = divmod(c, G)
        in_maps.append(
            {
                "xT": xTs[b],
                "wq": wqs[g],
                "wk": wks[g],
                "wv": wvs[g],
                "wo": wos[g],
                "bqk": bqks[g],
            }
        )
    return in_maps


_CACHE = {}


def _get_nc():
    if "nc" not in _CACHE:
        _CACHE["nc"] = build_nc()
    return _CACHE["nc"]


def run(inputs, trace=False):
    """Run the SPMD kernel; returns (full_output, BassKernelResults)."""
    inputs = {k: np.asarray(v) for k, v in inputs.items()}
    nc = _get_nc()
    in_maps = shard_inputs(**inputs)
    res = run_bass_kernel_spmd(
        nc, in_maps, core_ids=list(range(NCORES)), trace=trace
    )
    Wo = np.asarray(inputs["Wo"], np.float32)
    const_row = (
        np.asarray(inputs["bv"], np.float32) @ Wo + np.asarray(inputs["bo"], np.float32)
    )
    out = np.empty((B, S, DM), np.float32)
    for b in range(B):
        out[b] = (
            res.results[G * b]["out"].astype(np.float32)
            + res.results[G * b + 1]["out"].astype(np.float32)
            + const_row
        )
    return out, res


def kernel(**inputs):
    out, _ = run(inputs, trace=False)
    return out


# revision 55
# speedup vs baseline: 1.0059x; 1.0016x over previous
"""Causal self-attention Trainium2 kernel (8 NeuronCores, SPMD) — v3.

Sharding: 8 cores = 4 batches x 2 head-groups. Each core computes, for its
(batch b, head-group g): Q/K/V projections restricted to g's 8 heads
(column-parallel), causal attention for those heads, and the partial output
projection ctx_g @ Wo[g rows] (row-parallel). Host sums the two partials per
batch and adds the bias terms (bv @ Wo + bo).

v3 changes vs v2 (v2 measured 634 us, PE busy 91%, ~58 us PE idle; v3
measures ~627.5 us, P1/V phase fully PE-dense at 99-100%):
- P1/P1b fusion: Q/K rounds and V seq-tile rounds share one 6-bank PSUM pool
  with V rounds interleaved between Q/K rounds, so there is no P1->P1b
  transition stall. Head 0 runs a special 8-bank q+k-interleaved round
  (borrowing the p4s banks, idle during P1) so its chunk consumption
  (1.7us/chunk) stays above the x DMA delivery rate (~1.4us/chunk) — the PE
  is work-limited from the first matmul on.
- DMA head: x moves as 16 full chunks on SWDGE (issue-rate bound), first
  strips by need-order; deadlock-aware strip ordering (h1 strips after all x
  issues: they reuse h0's slots and would block the gpsimd queue).
- Head-tail software pipelining: each head's closing ops are deferred into
  the NEXT head's kt loop, split in two (final PV at kt==3; normalizer
  matmul + reciprocal + ctx mul at kt==5, with ctx normalized STRAIGHT from
  the pv PSUM bank — no pv->SBUF copy), so the PE never sits on the
  score->exp->acc dependency chains at head boundaries.
- Deficit-paced out-projection interleave: scalar exp is the local pacer in
  later q-blocks; p4 pops are paced by the per-q-block scalar-PE deficit.
  The qb3 Q projection of heads 5-7 is deferred out of P1 (re-reading x cols
  and wq strips from DRAM) and injected as filler for qb0, which otherwise
  has no backlog.
- Engine placement tuned by trace: exp + Q/K bias on scalar only, PSUM
  evictions and all normalization on DVE (cross-engine offloads of
  chain-adjacent ops to gpsimd measurably backfire on semaphore latency),
  out stores on the sync HWDGE ring (scalar helps only in the drain), SWDGE
  queues empty at kernel end (no long GpSimd drain).
- Output is written in bf16 (host upcasts and sums the two group partials).
"""

import sys

sys.path.insert(0, "/opt/trn_rl_repo")

from collections import deque
from contextlib import ExitStack

import numpy as np

import concourse.bass as bass
import concourse.tile as tile
from concourse import mybir
from concourse.bass_utils import run_bass_kernel_spmd

BF16 = mybir.dt.bfloat16
F32 = mybir.dt.float32
NP_BF16 = mybir.dt.np(BF16)

# Problem constants (hardcoded per contract).
B = 4          # batch
S = 2048       # sequence length
DM = 2048      # d_model
H = 16         # total heads
HD = 128       # head dim
G = 2          # head groups (tensor parallel degree)
NHL = H // G   # local heads per core
DHL = NHL * HD # local head dims
NCORES = 8
P = 128        # partitions
FD = 512       # matmul moving free dim (one PSUM bank of f32)
NKC = DM // P  # contraction chunks for projections
NST = S // P   # seq tiles (k tiles)
NQB = S // FD  # 512-wide q blocks
SCALE = 1.0 / float(np.sqrt(HD))
MASK_VAL = -1e30

# Per-head scalar-vs-PE deficit (ns) per q-block: how much PE filler each
# head's attention needs so the PE does not outrun the exp stream and stall
# on the score->exp->PV chain. Statically derived from measured ACTIVATE
# cost (259 ns + 0.836 ns/col) vs matmul cost (213 ns / 512 cols).
DEFICIT_NS = {0: 2100.0, 1: 2500.0, 2: 3400.0, 3: 5000.0}
MM_COST = 426.0  # one p4 micro-op = 2 matmuls of 512 rows
# Heads whose Q projection for the last q-block is deferred out of P1 and
# injected as PE filler during qb0 (which otherwise has no pop backlog).
DEFQ_HEADS = (5, 6, 7)

_WAIT_EXEMPT = {
    "NoOp",
    "EventSemaphore",
    "UnconditionalBranch",
    "RegisterMove",
    "TileRelease",
}


def _fix_sync_waits(nc, max_waits=1):
    """Hoist extra sync-waits onto single-wait NoOps on the issuing engine.

    Several walrus instruction encodings (PSEUDO_DMA_DIRECT2D, S3_LW, CTRL_NO,
    ...) have a single sync-wait slot and fail codegen with "Too many sync
    wait commands" when Tile attaches more. A NoOp on the same engine
    immediately before the instruction performs the extra wait at the
    sequencer, which is semantically identical.
    """
    f = nc.m.functions[0]
    fixed = 0

    def walk(blocks):
        nonlocal fixed
        for b in blocks:
            il = b.instructions
            i = 0
            while i < len(il):
                inst = il[i]
                si = getattr(inst, "sync_info", None)
                ow = list(si.on_wait) if si is not None and si.on_wait else []
                if inst.opcode not in _WAIT_EXEMPT and len(ow) > max_waits:
                    keep = ow[len(ow) - max_waits :]
                    extra = ow[: len(ow) - max_waits]
                    for j, w in enumerate(extra):
                        nop = mybir.InstNoOp(
                            name=f"{inst.name}_waitfix{j}",
                            engine=inst.engine,
                            ins=[],
                            outs=[],
                            bass_nofuse=True,
                            sync_info=mybir.SyncInfo(on_wait=[w], on_update=[]),
                        )
                        il.insert(i, nop)
                        i += 1
                    inst.sync_info = mybir.SyncInfo(
                        on_wait=keep,
                        on_update=list(si.on_update) if si.on_update else [],
                    )
                    fixed += 1
                i += 1
            walk(getattr(b, "blocks", []) or [])

    walk(f.blocks)
    return fixed


def build_nc(fix_waits=True):
    """Build the single-core Bass program (same program for all 8 cores)."""
    nc = bass.Bass()
    # Inputs are pre-arranged on the host so every DMA line is contiguous.
    # wq/wk are half-strip-major so one [P, NKC//2, P] half is a contiguous
    # 2 KB line per partition (256 B lines are below SDMA line rate).
    xT_d = nc.dram_tensor("xT", [P, NKC, S], BF16, kind="ExternalInput")
    wq_d = nc.dram_tensor("wq", [NHL, 2, P, NKC // 2, P], BF16, kind="ExternalInput")
    wk_d = nc.dram_tensor("wk", [NHL, 2, P, NKC // 2, P], BF16, kind="ExternalInput")
    wv_d = nc.dram_tensor("wv", [P, NKC, DHL], BF16, kind="ExternalInput")
    wo_d = nc.dram_tensor("wo", [P, DHL // P, DM], BF16, kind="ExternalInput")
    bqk_d = nc.dram_tensor("bqk", [P, 2, NHL], F32, kind="ExternalInput")
    out_d = nc.dram_tensor("out", [S, DM], BF16, kind="ExternalOutput")

    with tile.TileContext(nc) as tc:
        # ------------------------- pools (left stack) ---------------------
        es_main = ExitStack()
        consts = es_main.enter_context(tc.tile_pool(name="consts", bufs=1))
        bqk_sb = consts.tile([P, 2, NHL], F32)
        ones_sb = consts.tile([P, P], BF16)
        umask = consts.tile([P, P], F32)

        qkv = es_main.enter_context(tc.tile_pool(name="qkv", bufs=1))
        QT = qkv.tile([P, NHL, S], BF16)
        KT = qkv.tile([P, NHL, S], BF16)

        es_x = ExitStack()
        xpool = es_x.enter_context(tc.tile_pool(name="xpool", bufs=1))
        xT = xpool.tile([P, NKC, S], BF16)

        # ------------------------- pools (right stack) --------------------
        # LIFO close order: strips (end P1) -> wv (end P1b) -> V (end).
        es_v = ExitStack()
        vpool = es_v.enter_context(tc.tile_pool(name="vpool", bufs=1, side="right"))
        V = vpool.tile([P, NST, DHL], BF16)

        es_wv = ExitStack()
        wvpool = es_wv.enter_context(
            tc.tile_pool(name="wvpool", bufs=1, side="right")
        )
        wv_sb = wvpool.tile([P, NKC, DHL], BF16)

        es_strip = ExitStack()
        spool = es_strip.enter_context(
            tc.tile_pool(name="spool", bufs=6, side="right")
        )

        # ------------------------- DMA issue (order = priority) -----------
        # The SWDGE (gpsimd) ring spreads consecutive dma_starts across ~16
        # parallel queue rows. SWDGE issue costs ~0.6us per dma_start; x goes
        # first, split per (chunk, seq-half): 32 issues. The first q/k strips
        # ride the two HWDGE rings (issued by the otherwise idle sync and
        # scalar engines) so the first P1 matmul starts at ~2us.
        strips = {}  # (h, 'q'|'k', half) -> tile

        def load_strip(h, eng):
            for kind, src in (("q", wq_d), ("k", wk_d)):
                for half in range(2):
                    t = spool.tile(
                        [P, NKC // 2, P], BF16, tag="strip", name=f"w{kind}{h}_{half}"
                    )
                    eng.dma_start(out=t[:, :, :], in_=src[h, half, :, :, :])
                    strips[(h, kind, half)] = t

        nc.sync.dma_start(out=bqk_sb[:, :, :], in_=bqk_d[:, :, :])
        # Everything on SWDGE (per-transfer packets spread over all 16 DMA
        # engines; the aggregate ~360 GB/s HBM rate is the wall, so order
        # strictly by need). x moves as FULL chunks (4 KB lines, one issue
        # each). P1's h0 interleaves q AND k per chunk (1.7 us/chunk work vs
        # ~1.4 us/chunk delivery), so the PE is work-limited from the first
        # matmul on.
        def strip_piece(h, kind, half, eng):
            src = wq_d if kind == "q" else wk_d
            t = spool.tile(
                [P, NKC // 2, P], BF16, tag="strip", name=f"w{kind}{h}_{half}"
            )
            eng.dma_start(out=t[:, :, :], in_=src[h, half, :, :, :])
            strips[(h, kind, half)] = t

        nc.gpsimd.dma_start(out=xT[:, 0, :], in_=xT_d[:, 0, :])
        strip_piece(0, "q", 0, nc.gpsimd)
        strip_piece(0, "k", 0, nc.gpsimd)
        for i in range(1, NKC):
            nc.gpsimd.dma_start(out=xT[:, i, :], in_=xT_d[:, i, :])
            if i == 4:
                strip_piece(0, "q", 1, nc.gpsimd)
                strip_piece(0, "k", 1, nc.gpsimd)
        # h1 strips AFTER all x issues: their DMAs reuse h0's strip slots and
        # wait on h0 consumption — anything queued behind them on the gpsimd
        # engine would deadlock against P1's x needs.
        load_strip(1, nc.gpsimd)
        for j in range(0, NKC, 4):
            nc.gpsimd.dma_start(
                out=wv_sb[:, j : j + 4, :], in_=wv_d[:, j : j + 4, :]
            )

        # ------------------------- constants setup ------------------------
        nc.vector.memset(ones_sb[:, :], 1.0)
        # umask[k, q] = 0 if q >= k else MASK_VAL (transposed diagonal block).
        nc.gpsimd.memset(umask[:, :], 0.0)
        nc.gpsimd.affine_select(
            out=umask[:, :],
            in_=umask[:, :],
            compare_op=mybir.AluOpType.is_ge,
            fill=MASK_VAL,
            base=0,
            pattern=[[1, P]],
            channel_multiplier=-1,
        )

        # ------------------------- P1 + P1b: projections -------------------
        # One shared 6-bank PSUM pool for Q/K rounds (4 tiles) and V rounds
        # (2 tiles): V seq-tile rounds are interleaved between Q/K rounds
        # (one per round once x/wv are resident), so there is no P1->P1b
        # transition stall. p4s (banks 6-7, program-lifetime, right stack) is
        # idle during P1 and lends its 2 banks to h0's 8-bank q+k round.
        es_pp = ExitStack()
        ppsum = es_pp.enter_context(tc.tile_pool(name="ppsum", bufs=6, space="PSUM"))
        p4s = es_main.enter_context(
            tc.tile_pool(name="p4s", bufs=2, space="PSUM", side="right")
        )

        # h0: q and k interleaved per chunk, paced to x chunk arrival (the
        # x stream is still in flight; a q-only round would starve the PE).
        ps_q0 = [
            ppsum.tile([P, FD], F32, tag="pp", bufs=6, name=f"ppq0_{qb}")
            for qb in range(NQB)
        ]
        ps_k0 = [
            ppsum.tile([P, FD], F32, tag="pp", bufs=6, name=f"ppk0_{qb}")
            for qb in range(2)
        ] + [
            p4s.tile([P, FD], F32, tag="p4", bufs=2, name=f"ppk0_{qb}")
            for qb in (2, 3)
        ]
        for c in range(NKC):
            wq0 = strips[(0, "q", c // (NKC // 2))]
            wk0 = strips[(0, "k", c // (NKC // 2))]
            for qb in range(NQB):
                nc.tensor.matmul(
                    ps_q0[qb][:, :],
                    wq0[:, c % (NKC // 2), :],
                    xT[:, c, qb * FD : (qb + 1) * FD],
                    start=(c == 0),
                    stop=(c == NKC - 1),
                )
                nc.tensor.matmul(
                    ps_k0[qb][:, :],
                    wk0[:, c % (NKC // 2), :],
                    xT[:, c, qb * FD : (qb + 1) * FD],
                    start=(c == 0),
                    stop=(c == NKC - 1),
                )
        for qb in range(NQB):
            nc.scalar.activation(
                QT[:, 0, qb * FD : (qb + 1) * FD],
                ps_q0[qb][:, :],
                mybir.ActivationFunctionType.Identity,
                bias=bqk_sb[:, 0, 0:1],
            )
            nc.scalar.activation(
                KT[:, 0, qb * FD : (qb + 1) * FD],
                ps_k0[qb][:, :],
                mybir.ActivationFunctionType.Identity,
                bias=bqk_sb[:, 1, 0:1],
            )

        def v_round(st):
            ps = [
                ppsum.tile([P, FD], F32, tag="pp", bufs=6, name=f"vp{st}_{dc}")
                for dc in range(2)
            ]
            for c in range(NKC):
                for dc in range(2):
                    nc.tensor.matmul(
                        ps[dc][:, :],
                        xT[:, c, st * P : (st + 1) * P],
                        wv_sb[:, c, dc * FD : (dc + 1) * FD],
                        start=(c == 0),
                        stop=(c == NKC - 1),
                    )
            for dc in range(2):
                nc.vector.tensor_copy(V[:, st, dc * FD : (dc + 1) * FD], ps[dc][:, :])

        ri = 0
        vst = 0
        for h in range(1, NHL):
            if 2 <= h + 1 < NHL:
                load_strip(h + 1, nc.gpsimd)
            for kind in ("q", "k"):
                # Deferred-Q heads skip their last q-block here; it is
                # recomputed from a DRAM re-read as qb0 attention filler.
                qbs = range(3) if (kind == "q" and h in DEFQ_HEADS) else range(NQB)
                ps = {
                    qb: ppsum.tile(
                        [P, FD], F32, tag="pp", bufs=6, name=f"pp{kind}{h}_{qb}"
                    )
                    for qb in qbs
                }
                for c in range(NKC):
                    w = strips[(h, kind, c // (NKC // 2))]
                    for qb in qbs:
                        nc.tensor.matmul(
                            ps[qb][:, :],
                            w[:, c % (NKC // 2), :],
                            xT[:, c, qb * FD : (qb + 1) * FD],
                            start=(c == 0),
                            stop=(c == NKC - 1),
                        )
                dst = QT if kind == "q" else KT
                bias = bqk_sb[:, 0 if kind == "q" else 1, h : h + 1]
                for qb in qbs:
                    nc.scalar.activation(
                        dst[:, h, qb * FD : (qb + 1) * FD],
                        ps[qb][:, :],
                        mybir.ActivationFunctionType.Identity,
                        bias=bias,
                    )
                # Interleave one V seq-tile round once x and wv are resident.
                if ri >= 3 and vst < NST:
                    v_round(vst)
                    vst += 1
                ri += 1
        es_strip.close()
        while vst < NST:
            v_round(vst)
            vst += 1
        es_pp.close()
        es_wv.close()
        es_x.close()

        # ------------------------- attention + out-proj -------------------
        # wo goes into the SBUF freed by xT (left stack, after es_x.close()).
        es_attn = ExitStack()
        # x columns of the last q-block + the DEFQ heads' wq strips, re-read
        # from DRAM for the deferred-Q filler (keeping xT/strips resident
        # through attention would not fit SBUF).
        xqpool = es_attn.enter_context(tc.tile_pool(name="xqpool", bufs=1))
        xq3 = xqpool.tile([P, NKC, FD], BF16)
        rstrips = {}
        for h in DEFQ_HEADS:
            for half in range(2):
                rstrips[(h, half)] = xqpool.tile(
                    [P, NKC // 2, P], BF16, name=f"rwq{h}_{half}"
                )
        # Need-order: the first pops touch rstrips[h5] + xq3 chunk 0 first.
        nc.gpsimd.dma_start(out=rstrips[(5, 0)][:, :, :], in_=wq_d[5, 0, :, :, :])
        for i in range(0, NKC, 4):
            nc.gpsimd.dma_start(
                out=xq3[:, i : i + 4, :], in_=xT_d[:, i : i + 4, 3 * FD : S]
            )
            if i == 0:
                nc.gpsimd.dma_start(
                    out=rstrips[(5, 1)][:, :, :], in_=wq_d[5, 1, :, :, :]
                )
        for h in (6, 7):
            for half in range(2):
                nc.gpsimd.dma_start(
                    out=rstrips[(h, half)][:, :, :], in_=wq_d[h, half, :, :, :]
                )
        wopool = es_attn.enter_context(tc.tile_pool(name="wopool", bufs=1))
        wo_sb = wopool.tile([P, DHL // P, DM], BF16)
        for i in range(0, DHL // P, 4):
            nc.gpsimd.dma_start(
                out=wo_sb[:, i : i + 4, :], in_=wo_d[:, i : i + 4, :]
            )

        epool = es_attn.enter_context(tc.tile_pool(name="epool", bufs=5))
        apool = es_attn.enter_context(tc.tile_pool(name="apool", bufs=2))
        rpool = es_attn.enter_context(tc.tile_pool(name="rpool", bufs=2))
        cpool = es_attn.enter_context(tc.tile_pool(name="cpool", bufs=2))
        stpool = es_attn.enter_context(tc.tile_pool(name="stpool", bufs=2))
        # PSUM: sps/pvs/bcs on freed ppsum banks (0-5); p4s on banks 6-7,
        # which no earlier pool ever touched.
        sps = es_attn.enter_context(tc.tile_pool(name="sps", bufs=3, space="PSUM"))
        pvs = es_attn.enter_context(tc.tile_pool(name="pvs", bufs=2, space="PSUM"))
        bcs = es_attn.enter_context(tc.tile_pool(name="bcs", bufs=1, space="PSUM"))

        # Pending out-projection micro-ops with PE-cost and generation
        # tags, popped into attention kt slots by the deficit pacer.
        p4q = deque()  # items: (pe_cost_ns, fn, gen)

        def queue_p4(qb, ctx):
            for stl in range(4):
                st = qb * 4 + stl
                box = {}

                def alloc(box=box, st=st):
                    box["stage"] = stpool.tile(
                        [P, DM], BF16, tag="stage", name=f"stage{st}"
                    )

                p4q.append((0.0, alloc, qb))
                for half in range(2):

                    def mk_ps(box=box, st=st, half=half):
                        box["ps"] = [
                            p4s.tile([P, FD], F32, tag="p4", bufs=2, name=f"o{st}_{half}_{m}")
                            for m in range(2)
                        ]

                    p4q.append((0.0, mk_ps, qb))
                    for dc in range(DHL // P):

                        def mm(box=box, stl=stl, half=half, dc=dc, ctx=ctx):
                            for m in range(2):
                                nc.tensor.matmul(
                                    box["ps"][m][:, :],
                                    ctx[:, dc, stl * P : (stl + 1) * P],
                                    wo_sb[:, dc, (half * 2 + m) * FD : (half * 2 + m + 1) * FD],
                                    start=(dc == 0),
                                    stop=(dc == DHL // P - 1),
                                )

                        p4q.append((MM_COST, mm, qb))

                    def evict_store(box=box, st=st, half=half):
                        for m in range(2):
                            mc = half * 2 + m
                            # PSUM eviction on DVE (GpSimd has no PSUM port;
                            # scalar must stay exp-only during attention but
                            # is idle in the drain, where it halves the final
                            # eviction latency).
                            if st >= 12 and mc % 2 == 1:
                                nc.scalar.copy(
                                    box["stage"][:, mc * FD : (mc + 1) * FD],
                                    box["ps"][m][:, :],
                                )
                            else:
                                nc.vector.tensor_copy(
                                    box["stage"][:, mc * FD : (mc + 1) * FD],
                                    box["ps"][m][:, :],
                                )
                            # Out stores on the HWDGE rings (SWDGE queues stay
                            # empty at kernel end — no long GpSimd drain).
                            # During attention sync-only (scalar = exp pacer);
                            # in the drain phase scalar is idle, so alternate
                            # rings to halve the final-store latency.
                            se = nc.scalar if (st >= 12 and mc % 2 == 1) else nc.sync
                            se.dma_start(
                                out=out_d[st * P : (st + 1) * P, mc * FD : (mc + 1) * FD],
                                in_=box["stage"][:, mc * FD : (mc + 1) * FD],
                            )

                    p4q.append((0.0, evict_store, qb))

        popped_cost = 0.0
        quota = 0.0

        def pop_until(target):
            nonlocal popped_cost
            while p4q and popped_cost < target:
                cost, fn, _ = p4q.popleft()
                fn()
                popped_cost += cost

        def pop_stale(max_gen):
            # ctx(qb) reuses ctx(qb-2)'s buffer (cpool bufs=2): everything
            # queued at generation qb-2 or earlier must be emitted before
            # qb's first ctx write, or the PE pipeline stalls on the reuse.
            nonlocal popped_cost
            while p4q and p4q[0][2] <= max_gen:
                cost, fn, _ = p4q.popleft()
                fn()
                popped_cost += cost

        # Deferred-Q filler: the qb3 Q projection of DEFQ_HEADS, queued ahead
        # of the out-projection so qb0 (which has no p4 backlog yet) has PE
        # work to hide its exp-chain latency behind.
        for h in DEFQ_HEADS:
            qbox = {}

            def qalloc(qbox=qbox, h=h):
                qbox["ps"] = p4s.tile([P, FD], F32, tag="p4", bufs=2, name=f"defq{h}")

            p4q.append((0.0, qalloc, -1))
            for c in range(NKC):

                def qmm(qbox=qbox, h=h, c=c):
                    w = rstrips[(h, c // (NKC // 2))]
                    nc.tensor.matmul(
                        qbox["ps"][:, :],
                        w[:, c % (NKC // 2), :],
                        xq3[:, c, :],
                        start=(c == 0),
                        stop=(c == NKC - 1),
                    )

                p4q.append((213.0, qmm, -1))

            def qevict(qbox=qbox, h=h):
                nc.scalar.activation(
                    QT[:, h, 3 * FD : S],
                    qbox["ps"][:, :],
                    mybir.ActivationFunctionType.Identity,
                    bias=bqk_sb[:, 0, h : h + 1],
                )

            p4q.append((0.0, qevict, -1))

        for qb in range(NQB):
            kmax = 4 * (qb + 1)
            pop_stale(qb - 2)
            ctx = cpool.tile([P, NHL, FD], BF16, tag="ctx", name=f"ctx{qb}")
            tail_a = None
            tail_b = None
            for h in range(NHL):
                acc = apool.tile([P, FD], BF16, tag="acc", name=f"acc{h}_{qb}")
                pv = pvs.tile([P, FD], F32, tag="pv", bufs=2, name=f"pv{h}_{qb}")
                exp_t = {}
                lo_of = {}
                for kt in range(kmax):
                    j = kt - 4 * qb
                    lo = max(j, 0) * P
                    lo_of[kt] = lo
                    sp = sps.tile([P, FD], F32, tag="sps", bufs=3, name=f"s{h}_{qb}_{kt}")
                    nc.tensor.matmul(
                        sp[:, lo:FD],
                        KT[:, h, kt * P : (kt + 1) * P],
                        QT[:, h, qb * FD + lo : (qb + 1) * FD],
                        start=True,
                        stop=True,
                    )
                    if j >= 0:
                        nc.vector.tensor_add(
                            sp[:, lo : lo + P], sp[:, lo : lo + P], umask[:, :]
                        )
                    ex = epool.tile([P, FD], BF16, tag="exp", name=f"e{h}_{qb}_{kt}")
                    nc.scalar.activation(
                        ex[:, lo:FD],
                        sp[:, lo:FD],
                        mybir.ActivationFunctionType.Exp,
                        scale=SCALE,
                    )
                    # Row-sum accumulation on DVE (GpSimd's software tensor
                    # ops are ~5x slower and serialize the per-head chain).
                    if kt == 0:
                        nc.vector.tensor_copy(acc[:, :], ex[:, :])
                    else:
                        nc.vector.tensor_add(
                            acc[:, lo:FD], acc[:, lo:FD], ex[:, lo:FD]
                        )
                    exp_t[kt] = ex
                    # Deficit-paced out-projection filler for qb-1, emitted
                    # BEFORE this slot's PV so the PV's exp dependency has pop
                    # work as latency cover. Head 0 of qb0 pops nothing: the
                    # deferred-Q re-reads (xq3/rstrips) may still be in
                    # flight right at attention start.
                    if not (qb == 0 and h <= 1):
                        quota += DEFICIT_NS[qb] / kmax
                    pop_until(quota)
                    if kt > 0:
                        pkt = kt - 1
                        plo = lo_of[pkt]
                        nc.tensor.matmul(
                            pv[:, plo:FD],
                            V[:, pkt, h * P : (h + 1) * P],
                            exp_t[pkt][:, plo:FD],
                            start=(pkt == 0),
                            stop=False,
                        )
                    # Previous head's tail lands here, split: the final PV
                    # (scalar-exp dependent) four slots in, the normalizer
                    # (DVE acc-chain dependent) six slots in.
                    if kt == min(3, kmax - 1) and tail_a is not None:
                        tail_a()
                        tail_a = None
                    if kt == min(5, kmax - 1) and tail_b is not None:
                        tail_b()
                        tail_b = None

                def mk_tails(
                    h=h,
                    qb=qb,
                    kmax=kmax,
                    acc=acc,
                    pv=pv,
                    ex=exp_t[kmax - 1],
                    plo=lo_of[kmax - 1],
                    ctx=ctx,
                ):
                    def ta():
                        nc.tensor.matmul(
                            pv[:, plo:FD],
                            V[:, kmax - 1, h * P : (h + 1) * P],
                            ex[:, plo:FD],
                            start=(kmax == 1),
                            stop=True,
                        )

                    def tb():
                        # Normalizer: partition-reduce + broadcast in one
                        # matmul.
                        bc = bcs.tile([P, FD], F32, tag="bc", bufs=1, name=f"bc{h}_{qb}")
                        nc.tensor.matmul(
                            bc[:, :], ones_sb[:, :], acc[:, :], start=True, stop=True
                        )
                        recip = rpool.tile([P, FD], F32, tag="recip", name=f"r{h}_{qb}")
                        nc.vector.reciprocal_approx_fast(out=recip[:, :], in_=bc[:, :])
                        # Normalize ctx straight from the pv PSUM bank (saves
                        # a [P,512] f32 DVE copy per head; the bank is freed
                        # here, still two heads before its next writer).
                        nc.vector.tensor_mul(ctx[:, h, :], pv[:, :], recip[:, :])

                    return ta, tb

                tail_a, tail_b = mk_tails()
            # Last head of the q-block: cover the exp latency with forced
            # pops (when backlog exists), then flush.
            quota = max(quota, popped_cost + 2 * MM_COST)
            pop_until(quota)
            tail_a()
            tail_b()
            tail_a = None
            tail_b = None
            queue_p4(qb, ctx)
        while p4q:
            cost, fn, _ = p4q.popleft()
            fn()
        es_attn.close()
        es_v.close()
        es_main.close()

    # Populate .instr bytes for the custom-DVE InstISA (reciprocal_approx) —
    # raw Bass skips this Bacc pass and the NEFF compiler rejects the empty
    # encoding with "ISA wrong length".
    mybir.codegen_inst_isa_subclasses(nc)
    if fix_waits:
        _fix_sync_waits(nc)
    return nc


def shard_inputs(x, Wq, bq, Wk, bk, Wv, bv, Wo, bo):
    """Host-side sharding: returns per-core input maps (bf16 pre-arranged)."""
    xTs = []
    for b in range(B):
        xt = np.ascontiguousarray(np.asarray(x)[b].T).astype(NP_BF16)  # [dm, seq]
        xTs.append(np.ascontiguousarray(xt.reshape(NKC, P, S).transpose(1, 0, 2)))
    wqs, wks, wvs, wos, bqks = [], [], [], [], []
    for g in range(G):
        sl = slice(g * DHL, (g + 1) * DHL)
        wq_s = np.asarray(Wq)[:, sl].astype(NP_BF16)
        wk_s = np.asarray(Wk)[:, sl].astype(NP_BF16)
        wv_s = np.asarray(Wv)[:, sl].astype(NP_BF16)
        wo_s = np.asarray(Wo)[sl, :].astype(NP_BF16)
        wqs.append(
            np.ascontiguousarray(
                wq_s.reshape(2, NKC // 2, P, NHL, P).transpose(3, 0, 2, 1, 4)
            )
        )
        wks.append(
            np.ascontiguousarray(
                wk_s.reshape(2, NKC // 2, P, NHL, P).transpose(3, 0, 2, 1, 4)
            )
        )
        wvs.append(np.ascontiguousarray(wv_s.reshape(NKC, P, DHL).transpose(1, 0, 2)))
        wos.append(
            np.ascontiguousarray(wo_s.reshape(DHL // P, P, DM).transpose(1, 0, 2))
        )
        bqk = np.stack(
            [
                np.asarray(bq, np.float32)[sl].reshape(NHL, P),
                np.asarray(bk, np.float32)[sl].reshape(NHL, P),
            ]
        )  # [2, nhl, P]
        bqks.append(np.ascontiguousarray(bqk.transpose(2, 0, 1)))  # [P, 2, nhl]
    in_maps = []
    for c in range(B * G):
        b, g = divmod(c, G)
        in_maps.append(
            {
                "xT": xTs[b],
                "wq": wqs[g],
                "wk": wks[g],
                "wv": wvs[g],
                "wo": wos[g],
                "bqk": bqks[g],
            }
        )
    return in_maps


_CACHE = {}


def _get_nc():
    if "nc" not in _CACHE:
        _CACHE["nc"] = build_nc()
    return _CACHE["nc"]


def run(inputs, trace=False):
    """Run the SPMD kernel; returns (full_output, BassKernelResults)."""
    inputs = {k: np.asarray(v) for k, v in inputs.items()}
    nc = _get_nc()
    in_maps = shard_inputs(**inputs)
    res = run_bass_kernel_spmd(
        nc, in_maps, core_ids=list(range(NCORES)), trace=trace
    )
    Wo = np.asarray(inputs["Wo"], np.float32)
    const_row = (
        np.asarray(inputs["bv"], np.float32) @ Wo + np.asarray(inputs["bo"], np.float32)
    )
    out = np.empty((B, S, DM), np.float32)
    for b in range(B):
        out[b] = (
            res.results[G * b]["out"].astype(np.float32)
            + res.results[G * b + 1]["out"].astype(np.float32)
            + const_row
        )
    return out, res


def kernel(**inputs):
    out, _ = run(inputs, trace=False)
    return out
